# revision 23
# baseline (speedup 1.0000x reference)
# Causal self-attention (B=4, T=2048, C=1024, 16 heads) on 8 NeuronCores.
# Sharding: core = (batch b = core//2) x (head-group hg = core%2, 8 heads each).
# Each core computes its 8 heads' attention for its batch plus the row-slice of
# the output projection; the host sums the two partial projections per batch.
#
# Self-contained: hardcodes shapes; builds + compiles the Bass program once.

import contextlib

import numpy as np
import ml_dtypes

import concourse.bass as bass
import concourse.tile as tile
from concourse import bacc, mybir
from concourse.bass import AP
from concourse.bass_utils import run_bass_kernel_spmd

F32 = mybir.dt.float32
BF16 = mybir.dt.bfloat16
EXP = mybir.ActivationFunctionType.Exp
IDENT = mybir.ActivationFunctionType.Identity

B, T, C = 4, 2048, 1024
NH, HS = 16, 64
NHPC = 8          # heads per core
D = NHPC * HS     # 512: per-core qkv width
NCORES = 8
TT = T // 128     # 16 token tiles
TC = T // 512     # 4 token chunks
CT = C // 128     # 8 contraction tiles
DT = D // 128     # 4 d-tiles of qT/kT (= head pairs)
VW = 68           # per-head stride in v tile: [v(64) | ones | pad3]

_cache = {}


def _bcast_row(ap, nrep):
    """AP reading a [1, N] slice nrep times (free-dim step-0 broadcast)."""
    return AP(ap.tensor, ap.offset, [ap.ap[0], [0, nrep]] + ap.ap[1:])


def _build():
    nc = bacc.Bacc("TRN2", target_bir_lowering=False, debug=False,
                   num_devices=NCORES)

    xT = nc.dram_tensor("xT", [C, T], BF16, kind="ExternalInput")
    wq = nc.dram_tensor("wq", [C, D], BF16, kind="ExternalInput")
    wk = nc.dram_tensor("wk", [C, D], BF16, kind="ExternalInput")
    wv = nc.dram_tensor("wv", [C, D], BF16, kind="ExternalInput")
    wp = nc.dram_tensor("wp", [D, C], BF16, kind="ExternalInput")
    bq = nc.dram_tensor("bq", [128, DT], F32, kind="ExternalInput")
    bk = nc.dram_tensor("bk", [128, DT], F32, kind="ExternalInput")
    bvbc = nc.dram_tensor("bvbc", [128, D], F32, kind="ExternalInput")
    ypT = nc.dram_tensor("ypT", [C, T], F32, kind="ExternalOutput")

    with tile.TileContext(nc) as tc, contextlib.ExitStack() as ctx:
        cpool = ctx.enter_context(tc.tile_pool(name="consts", bufs=1))
        xpool = ctx.enter_context(tc.tile_pool(name="x", bufs=1))
        wpool = ctx.enter_context(tc.tile_pool(name="w", bufs=1))
        qkpool = ctx.enter_context(tc.tile_pool(name="qk", bufs=1))
        vpool = ctx.enter_context(tc.tile_pool(name="v", bufs=1))
        opool = ctx.enter_context(tc.tile_pool(name="oT", bufs=1))
        ptpool = ctx.enter_context(tc.tile_pool(name="pt", bufs=7))
        ypool = ctx.enter_context(tc.tile_pool(name="y", bufs=3))
        avpool = ctx.enter_context(tc.tile_pool(name="av", bufs=3))
        rspool = ctx.enter_context(tc.tile_pool(name="rs", bufs=10))
        rvpool = ctx.enter_context(tc.tile_pool(name="rv", bufs=4))
        qkv_ps = ctx.enter_context(
            tc.tile_pool(name="qkv_ps", bufs=2, space="PSUM"))
        s_ps = ctx.enter_context(
            tc.tile_pool(name="s_ps", bufs=2, space="PSUM"))
        o_ps = ctx.enter_context(
            tc.tile_pool(name="o_ps", bufs=2, space="PSUM"))

        # ---- input DMAs (ordered by first use; spread over sync/scalar) ----
        wvt = [wpool.tile([128, D], BF16, tag=f"wv{i}", name=f"wv{i}")
               for i in range(CT)]
        for i in range(CT):
            (nc.sync if i % 2 == 0 else nc.scalar).dma_start(
                wvt[i][:], wv.ap()[i * 128:(i + 1) * 128, :])
        xt = [xpool.tile([128, T], BF16, tag=f"xT{i}", name=f"xT{i}")
              for i in range(CT)]
        for cch in range(TC):
            for i in range(CT):
                (nc.sync if i % 2 == 0 else nc.scalar).dma_start(
                    xt[i][:, cch * 512:(cch + 1) * 512],
                    xT.ap()[i * 128:(i + 1) * 128, cch * 512:(cch + 1) * 512])
        wqt = [wpool.tile([128, D], BF16, tag=f"wq{i}", name=f"wq{i}")
               for i in range(CT)]
        wkt = [wpool.tile([128, D], BF16, tag=f"wk{i}", name=f"wk{i}")
               for i in range(CT)]
        for i in range(CT):
            nc.sync.dma_start(wqt[i][:], wq.ap()[i * 128:(i + 1) * 128, :])
            nc.scalar.dma_start(wkt[i][:], wk.ap()[i * 128:(i + 1) * 128, :])
        wpt = [wpool.tile([128, C], BF16, tag=f"wp{i}", name=f"wp{i}")
               for i in range(DT)]
        for i in range(DT):
            nc.sync.dma_start(wpt[i][:], wp.ap()[i * 128:(i + 1) * 128, :])
        bqt = cpool.tile([128, DT], F32, tag="bq")
        bkt = cpool.tile([128, DT], F32, tag="bk")
        bvt = cpool.tile([128, D], F32, tag="bv")
        nc.sync.dma_start(bqt[:], bq.ap())
        nc.scalar.dma_start(bkt[:], bk.ap())
        nc.sync.dma_start(bvt[:], bvbc.ap())

        # ---- one-time consts ----
        ones8 = cpool.tile([128, NHPC], F32, tag="ones8")
        nc.vector.memset(ones8[:], 1.0)
        ones_t = cpool.tile([128, 512], F32, tag="ones_t")
        nc.vector.memset(ones_t[:], 1.0)
        masks = []
        for t in range(4):
            mf = cpool.tile([128, 512], F32, tag=f"mf{t}", name=f"mf{t}")
            nc.gpsimd.affine_select(
                mf[:], ones_t[:], pattern=[[1, 512]],
                compare_op=mybir.AluOpType.is_ge, fill=0.0,
                base=-(128 * t), channel_multiplier=-1)
            mb = cpool.tile([128, 512], BF16, tag=f"mb{t}", name=f"mb{t}")
            nc.vector.tensor_copy(mb[:], mf[:])
            masks.append(mb)

        # ---- v = x @ Wv + bv, token-major, [v|ones] per head ----
        # warm-up matmuls: keep PE busy (and HAM warm) during input DMA
        warm_ps = qkv_ps.tile([128, 512], F32, tag="qkv", name="warmps")
        for w in range(24):
            nc.tensor.matmul(
                warm_ps[:], wvt[0][:, 0:128], wvt[0][:, 0:512],
                start=True, stop=True, skip_group_check=True)

        vt = [vpool.tile([128, NHPC * VW], BF16, tag=f"v{j}", name=f"v{j}")
              for j in range(TT)]

        def emit_v(j):
            ps = qkv_ps.tile([128, D], F32, tag="qkv", name="qkvps")
            for ct in range(CT):
                nc.tensor.matmul(
                    ps[:], xt[ct][:, j * 128:(j + 1) * 128], wvt[ct][:],
                    start=(ct == 0), stop=(ct == CT - 1))
            vre = vt[j][:].rearrange("p (h x) -> p h x", h=NHPC)
            nc.vector.tensor_copy(
                vre[:, :, 0:1], ones8[:].rearrange("p (h x) -> p h x", x=1))
            nc.vector.tensor_add(
                vre[:, :, 1:65],
                ps[:].rearrange("p (h x) -> p h x", h=NHPC),
                bvt[:].rearrange("p (h x) -> p h x", h=NHPC))

        # ---- qT/kT/oT tiles ----
        qT = [qkpool.tile([128, T], BF16, tag=f"q{d}", name=f"q{d}")
              for d in range(DT)]
        kT = [qkpool.tile([128, T], BF16, tag=f"k{d}", name=f"k{d}")
              for d in range(DT)]
        oT = [opool.tile([128, T], BF16, tag=f"oT{d}", name=f"oT{d}")
              for d in range(DT)]

        def emit_qk_group(hp, idx):
            c, which = idx // 2, idx % 2
            wt_, bt_, out = ((wqt, bqt, qT), (wkt, bkt, kT))[which]
            ps = qkv_ps.tile([128, 512], F32, tag="qkv", name="qkvps")
            for ct in range(CT):
                nc.tensor.matmul(
                    ps[:], wt_[ct][:, hp * 128:(hp + 1) * 128],
                    xt[ct][:, c * 512:(c + 1) * 512],
                    start=(ct == 0), stop=(ct == CT - 1))
            nc.scalar.activation(
                out[hp][:, c * 512:(c + 1) * 512], ps[:], IDENT,
                bias=bt_[:, hp:hp + 1])

        def emit_proj(c):
            for o in range(CT):
                ps = qkv_ps.tile([128, 512], F32, tag="qkv", name="qkvps")
                for hp in range(DT):
                    nc.tensor.matmul(
                        ps[:], wpt[hp][:, o * 128:(o + 1) * 128],
                        oT[hp][:, c * 512:(c + 1) * 512],
                        start=(hp == 0), stop=(hp == DT - 1))
                ys = ypool.tile([128, 512], F32, tag="y", name="ys")
                nc.vector.tensor_copy(ys[:], ps[:])
                nc.sync.dma_start(
                    ypT.ap()[o * 128:(o + 1) * 128, c * 512:(c + 1) * 512],
                    ys[:])

        # deferred normalization: scale oT chunk piece by 1/rowsum
        def emit_norm(hp, c, rv0, rv1):
            cs = slice(c * 512, (c + 1) * 512)
            nc.vector.reciprocal_approx_fast(rv0[0:64, :], rv0[0:64, :])
            nc.vector.tensor_mul(
                oT[hp][0:64, cs], oT[hp][0:64, cs], rv0[0:64, :])
            nc.vector.reciprocal_approx_fast(rv1[:, :], rv1[:, :])
            nc.vector.tensor_mul(
                oT[hp][64:128, cs], oT[hp][64:128, cs], rv1[64:128, :])
            if hp == DT - 1:
                emit_proj(c)

        pending = []

        def flush_pending():
            while pending:
                emit_norm(*pending.pop(0))

        # ---- attention per head pair; qk of next pair interleaved ----
        for hp in range(DT):
            for c in range(TC):
                if hp == 0:
                    for j in range(4 * c, 4 * c + 4):
                        emit_v(j)
                    emit_qk_group(0, 2 * c)
                    emit_qk_group(0, 2 * c + 1)
                    if c >= 2:
                        for g in range(4 * (c - 2), 4 * (c - 2) + 4):
                            emit_qk_group(1, g)
                elif hp + 1 < DT:
                    emit_qk_group(hp + 1, 2 * c)
                    emit_qk_group(hp + 1, 2 * c + 1)
                njt = 4 * c + 4
                op0 = o_ps.tile([128, 512], F32, tag="o", name="ops")
                op1 = o_ps.tile([128, 512], F32, tag="o", name="ops")
                for jt in range(njt):
                    sp = s_ps.tile([128, 1024], F32, tag="s", name="sps")
                    for half in range(2):
                        nc.tensor.matmul(
                            sp[:, half * 512:(half + 1) * 512],
                            kT[hp][half * 64:(half + 1) * 64,
                                   jt * 128:(jt + 1) * 128],
                            qT[hp][half * 64:(half + 1) * 64,
                                   c * 512:(c + 1) * 512],
                            start=True, stop=True)
                    pt = ptpool.tile([128, 1024], BF16, tag="pt", name="pt")
                    nc.scalar.activation(pt[:], sp[:], EXP, scale=0.125)
                    if jt >= 4 * c:
                        t = jt - 4 * c
                        nc.vector.tensor_mul(
                            pt[:, 0:512], pt[:, 0:512], masks[t][:])
                        pv = pt[:, 512:1024]
                        nc.gpsimd.affine_select(
                            pv, pv, pattern=[[1, 512]],
                            compare_op=mybir.AluOpType.is_ge, fill=0.0,
                            base=-(128 * t), channel_multiplier=-1)
                    for half, op in ((0, op0), (1, op1)):
                        h = 2 * hp + half
                        nc.tensor.matmul(
                            op[0:65, :], vt[jt][:, h * VW:h * VW + 65],
                            pt[:, half * 512:(half + 1) * 512],
                            start=(jt == 0), stop=(jt == njt - 1))

                # stage unnormalized O + rowsums out of PSUM (fast release)
                cs = slice(c * 512, (c + 1) * 512)
                rs0 = rspool.tile([128, 512], F32, tag="rs", name="rs")
                rs1 = rspool.tile([128, 512], F32, tag="rs", name="rs")
                av0 = avpool.tile([128, 512], BF16, tag="av", name="av")
                av1 = avpool.tile([128, 512], BF16, tag="av", name="av")
                nc.vector.tensor_copy(av0[0:64, :], op0[0:64, :])
                nc.vector.tensor_copy(av0[64:65, :], op0[64:65, :])
                nc.vector.tensor_copy(rs0[0:1, :], op0[0:1, :])
                nc.vector.tensor_copy(av1[0:64, :], op1[0:64, :])
                nc.vector.tensor_copy(av1[64:65, :], op1[64:65, :])
                nc.vector.tensor_copy(rs1[0:1, :], op1[0:1, :])
                nc.sync.dma_start(oT[hp][0:64, cs], av0[1:65, :])
                nc.sync.dma_start(oT[hp][64:128, cs], av1[1:65, :])
                # rowsum broadcast via gpsimd (fast, no DMA round-trip)
                rv0 = rvpool.tile([128, 512], F32, tag="rv", name="rv")
                rv1 = rvpool.tile([128, 512], F32, tag="rv", name="rv")
                nc.gpsimd.partition_broadcast(rv0[:, :], rs0[0:1, :])
                nc.gpsimd.partition_broadcast(rv1[:, :], rs1[0:1, :])

                # run the PREVIOUS chunk's reciprocal+scale (deps long done)
                flush_pending()
                pending.append((hp, c, rv0, rv1))
        flush_pending()

    nc.compile()
    return nc


def _shard_inputs(x, Wk, bk, Wq, bq, Wv, bv, Wp, bp):
    bf = ml_dtypes.bfloat16
    in_maps = []
    for core in range(NCORES):
        b, hg = core // 2, core % 2
        sl = slice(hg * D, (hg + 1) * D)
        in_maps.append({
            "xT": np.ascontiguousarray(x[b].T).astype(bf),
            "wq": np.ascontiguousarray(Wq[:, sl]).astype(bf),
            "wk": np.ascontiguousarray(Wk[:, sl]).astype(bf),
            "wv": np.ascontiguousarray(Wv[:, sl]).astype(bf),
            "wp": np.ascontiguousarray(Wp[sl, :]).astype(bf),
            "bq": np.ascontiguousarray(
                bq[sl].reshape(DT, 128).T).astype(np.float32),
            "bk": np.ascontiguousarray(
                bk[sl].reshape(DT, 128).T).astype(np.float32),
            "bvbc": np.ascontiguousarray(
                np.broadcast_to(bv[sl], (128, D))).astype(np.float32),
        })
    return in_maps


def kernel(x, Wk, bk, Wq, bq, Wv, bv, Wp, bp, _trace=False, _trace_kwargs=None):
    x, Wk, bk, Wq, bq, Wv, bv, Wp, bp = [
        np.asarray(a) for a in (x, Wk, bk, Wq, bq, Wv, bv, Wp, bp)]
    if "nc" not in _cache:
        _cache["nc"] = _build()
    nc = _cache["nc"]
    in_maps = _shard_inputs(x, Wk, bk, Wq, bq, Wv, bv, Wp, bp)
    kw = dict(_trace_kwargs or {})
    res = run_bass_kernel_spmd(nc, in_maps, core_ids=list(range(NCORES)),
                               trace=_trace, **kw)
    out = np.empty((B, T, C), np.float32)
    for b in range(B):
        yp = res.results[2 * b]["ypT"] + res.results[2 * b + 1]["ypT"]
        out[b] = yp.T + bp[None, :]
    if _trace:
        _cache["last_results"] = res
    return out


# revision 25
# speedup vs baseline: 1.0171x; 1.0171x over previous
# Causal self-attention (B=4, T=2048, C=1024, 16 heads) on 8 NeuronCores.
# Sharding: core = (batch b = core//2) x (head-group hg = core%2, 8 heads each).
# Each core computes its 8 heads' attention for its batch plus the row-slice of
# the output projection; the host sums the two partial projections per batch.
#
# Self-contained: hardcodes shapes; builds + compiles the Bass program once.

import contextlib

import numpy as np
import ml_dtypes

import concourse.bass as bass
import concourse.tile as tile
from concourse import bacc, mybir
from concourse.bass import AP
from concourse.bass_utils import run_bass_kernel_spmd

F32 = mybir.dt.float32
BF16 = mybir.dt.bfloat16
EXP = mybir.ActivationFunctionType.Exp
IDENT = mybir.ActivationFunctionType.Identity

B, T, C = 4, 2048, 1024
NH, HS = 16, 64
NHPC = 8          # heads per core
D = NHPC * HS     # 512: per-core qkv width
NCORES = 8
TT = T // 128     # 16 token tiles
TC = T // 512     # 4 token chunks
CT = C // 128     # 8 contraction tiles
DT = D // 128     # 4 d-tiles of qT/kT (= head pairs)
VW = 68           # per-head stride in v tile: [v(64) | ones | pad3]

_cache = {}


def _bcast_row(ap, nrep):
    """AP reading a [1, N] slice nrep times (free-dim step-0 broadcast)."""
    return AP(ap.tensor, ap.offset, [ap.ap[0], [0, nrep]] + ap.ap[1:])


def _build():
    nc = bacc.Bacc("TRN2", target_bir_lowering=False, debug=False,
                   num_devices=NCORES)

    xT = nc.dram_tensor("xT", [C, T], BF16, kind="ExternalInput")
    wq = nc.dram_tensor("wq", [C, D], BF16, kind="ExternalInput")
    wk = nc.dram_tensor("wk", [C, D], BF16, kind="ExternalInput")
    wv = nc.dram_tensor("wv", [C, D], BF16, kind="ExternalInput")
    wp = nc.dram_tensor("wp", [D, C], BF16, kind="ExternalInput")
    bq = nc.dram_tensor("bq", [128, DT], F32, kind="ExternalInput")
    bk = nc.dram_tensor("bk", [128, DT], F32, kind="ExternalInput")
    bvbc = nc.dram_tensor("bvbc", [128, D], F32, kind="ExternalInput")
    ypT = nc.dram_tensor("ypT", [C, T], F32, kind="ExternalOutput")

    with tile.TileContext(nc) as tc, contextlib.ExitStack() as ctx:
        cpool = ctx.enter_context(tc.tile_pool(name="consts", bufs=1))
        xpool = ctx.enter_context(tc.tile_pool(name="x", bufs=1))
        wpool = ctx.enter_context(tc.tile_pool(name="w", bufs=1))
        qkpool = ctx.enter_context(tc.tile_pool(name="qk", bufs=1))
        vpool = ctx.enter_context(tc.tile_pool(name="v", bufs=1))
        opool = ctx.enter_context(tc.tile_pool(name="oT", bufs=1))
        ptpool = ctx.enter_context(tc.tile_pool(name="pt", bufs=8))
        ypool = ctx.enter_context(tc.tile_pool(name="y", bufs=4))
        avpool = ctx.enter_context(tc.tile_pool(name="av", bufs=3))
        rspool = ctx.enter_context(tc.tile_pool(name="rs", bufs=6))
        rvpool = ctx.enter_context(tc.tile_pool(name="rv", bufs=4))
        qkv_ps = ctx.enter_context(
            tc.tile_pool(name="qkv_ps", bufs=2, space="PSUM"))
        s_ps = ctx.enter_context(
            tc.tile_pool(name="s_ps", bufs=2, space="PSUM"))
        o_ps = ctx.enter_context(
            tc.tile_pool(name="o_ps", bufs=2, space="PSUM"))

        # ---- input DMAs (ordered by first use; spread over sync/scalar) ----
        wvt = [wpool.tile([128, D], BF16, tag=f"wv{i}", name=f"wv{i}")
               for i in range(CT)]
        for i in range(CT):
            (nc.sync if i % 2 == 0 else nc.scalar).dma_start(
                wvt[i][:], wv.ap()[i * 128:(i + 1) * 128, :])
        xt = [xpool.tile([128, T], BF16, tag=f"xT{i}", name=f"xT{i}")
              for i in range(CT)]
        for cch in range(TC):
            for i in range(CT):
                (nc.sync if i % 2 == 0 else nc.scalar).dma_start(
                    xt[i][:, cch * 512:(cch + 1) * 512],
                    xT.ap()[i * 128:(i + 1) * 128, cch * 512:(cch + 1) * 512])
        wqt = [wpool.tile([128, D], BF16, tag=f"wq{i}", name=f"wq{i}")
               for i in range(CT)]
        wkt = [wpool.tile([128, D], BF16, tag=f"wk{i}", name=f"wk{i}")
               for i in range(CT)]
        for i in range(CT):
            nc.sync.dma_start(wqt[i][:], wq.ap()[i * 128:(i + 1) * 128, :])
            nc.scalar.dma_start(wkt[i][:], wk.ap()[i * 128:(i + 1) * 128, :])
        wpt = [wpool.tile([128, C], BF16, tag=f"wp{i}", name=f"wp{i}")
               for i in range(DT)]
        for i in range(DT):
            nc.sync.dma_start(wpt[i][:], wp.ap()[i * 128:(i + 1) * 128, :])
        bqt = cpool.tile([128, DT], F32, tag="bq")
        bkt = cpool.tile([128, DT], F32, tag="bk")
        bvt = cpool.tile([128, D], F32, tag="bv")
        nc.sync.dma_start(bqt[:], bq.ap())
        nc.scalar.dma_start(bkt[:], bk.ap())
        nc.sync.dma_start(bvt[:], bvbc.ap())

        # ---- one-time consts ----
        ones8 = cpool.tile([128, NHPC], F32, tag="ones8")
        nc.vector.memset(ones8[:], 1.0)
        ones_t = cpool.tile([128, 512], F32, tag="ones_t")
        nc.vector.memset(ones_t[:], 1.0)
        masks = []
        for t in range(4):
            mf = cpool.tile([128, 512], F32, tag=f"mf{t}", name=f"mf{t}")
            nc.gpsimd.affine_select(
                mf[:], ones_t[:], pattern=[[1, 512]],
                compare_op=mybir.AluOpType.is_ge, fill=0.0,
                base=-(128 * t), channel_multiplier=-1)
            mb = cpool.tile([128, 512], BF16, tag=f"mb{t}", name=f"mb{t}")
            nc.vector.tensor_copy(mb[:], mf[:])
            masks.append(mb)

        # ---- v = x @ Wv + bv, token-major, [v|ones] per head ----
        # warm-up matmuls: keep PE busy (and HAM warm) during input DMA
        warm_ps = qkv_ps.tile([128, 512], F32, tag="qkv", name="warmps")
        for w in range(24):
            nc.tensor.matmul(
                warm_ps[:], wvt[0][:, 0:128], wvt[0][:, 0:512],
                start=True, stop=True, skip_group_check=True)

        vt = [vpool.tile([128, NHPC * VW], BF16, tag=f"v{j}", name=f"v{j}")
              for j in range(TT)]

        def emit_v(j):
            ps = qkv_ps.tile([128, D], F32, tag="qkv", name="qkvps")
            for ct in range(CT):
                nc.tensor.matmul(
                    ps[:], xt[ct][:, j * 128:(j + 1) * 128], wvt[ct][:],
                    start=(ct == 0), stop=(ct == CT - 1))
            vre = vt[j][:].rearrange("p (h x) -> p h x", h=NHPC)
            nc.vector.tensor_copy(
                vre[:, :, 0:1], ones8[:].rearrange("p (h x) -> p h x", x=1))
            nc.vector.tensor_add(
                vre[:, :, 1:65],
                ps[:].rearrange("p (h x) -> p h x", h=NHPC),
                bvt[:].rearrange("p (h x) -> p h x", h=NHPC))

        # ---- qT/kT/oT tiles ----
        qT = [qkpool.tile([128, T], BF16, tag=f"q{d}", name=f"q{d}")
              for d in range(DT)]
        kT = [qkpool.tile([128, T], BF16, tag=f"k{d}", name=f"k{d}")
              for d in range(DT)]
        oT = [opool.tile([128, T], BF16, tag=f"oT{d}", name=f"oT{d}")
              for d in range(DT)]

        def emit_qk_group(hp, idx):
            c, which = idx // 2, idx % 2
            wt_, bt_, out = ((wqt, bqt, qT), (wkt, bkt, kT))[which]
            ps = qkv_ps.tile([128, 512], F32, tag="qkv", name="qkvps")
            for ct in range(CT):
                nc.tensor.matmul(
                    ps[:], wt_[ct][:, hp * 128:(hp + 1) * 128],
                    xt[ct][:, c * 512:(c + 1) * 512],
                    start=(ct == 0), stop=(ct == CT - 1))
            nc.scalar.activation(
                out[hp][:, c * 512:(c + 1) * 512], ps[:], IDENT,
                bias=bt_[:, hp:hp + 1])

        def emit_proj(c):
            for o in range(CT):
                ps = qkv_ps.tile([128, 512], F32, tag="qkv", name="qkvps")
                for hp in range(DT):
                    nc.tensor.matmul(
                        ps[:], wpt[hp][:, o * 128:(o + 1) * 128],
                        oT[hp][:, c * 512:(c + 1) * 512],
                        start=(hp == 0), stop=(hp == DT - 1))
                ys = ypool.tile([128, 512], F32, tag="y", name="ys")
                nc.vector.tensor_copy(ys[:], ps[:])
                nc.sync.dma_start(
                    ypT.ap()[o * 128:(o + 1) * 128, c * 512:(c + 1) * 512],
                    ys[:])

        # deferred normalization: scale oT chunk piece by 1/rowsum
        def emit_norm(hp, c, rv0, rv1):
            cs = slice(c * 512, (c + 1) * 512)
            nc.vector.reciprocal_approx_fast(rv0[0:64, :], rv0[0:64, :])
            nc.vector.tensor_mul(
                oT[hp][0:64, cs], oT[hp][0:64, cs], rv0[0:64, :])
            nc.vector.reciprocal_approx_fast(rv1[:, :], rv1[:, :])
            nc.vector.tensor_mul(
                oT[hp][64:128, cs], oT[hp][64:128, cs], rv1[64:128, :])
            if hp == DT - 1:
                emit_proj(c)

        pending = []

        def flush_pending(keep=0):
            while len(pending) > keep:
                emit_norm(*pending.pop(0))

        # ---- attention per head pair; qk of next pair interleaved ----
        for j in range(TT):
            emit_v(j)
        for hp in range(DT):
            if hp == 0:
                for idx in range(8):
                    emit_qk_group(0, idx)
            for c in range(TC):
                if hp + 1 < DT:
                    emit_qk_group(hp + 1, 2 * c)
                    emit_qk_group(hp + 1, 2 * c + 1)
                njt = 4 * c + 4
                op0 = o_ps.tile([128, 512], F32, tag="o", name="ops")
                op1 = o_ps.tile([128, 512], F32, tag="o", name="ops")
                for jt in range(njt):
                    sp = s_ps.tile([128, 1024], F32, tag="s", name="sps")
                    for half in range(2):
                        nc.tensor.matmul(
                            sp[:, half * 512:(half + 1) * 512],
                            kT[hp][half * 64:(half + 1) * 64,
                                   jt * 128:(jt + 1) * 128],
                            qT[hp][half * 64:(half + 1) * 64,
                                   c * 512:(c + 1) * 512],
                            start=True, stop=True)
                    pt = ptpool.tile([128, 1024], BF16, tag="pt", name="pt")
                    nc.scalar.activation(pt[:], sp[:], EXP, scale=0.125)
                    if jt >= 4 * c:
                        t = jt - 4 * c
                        nc.vector.tensor_mul(
                            pt[:, 0:512], pt[:, 0:512], masks[t][:])
                        pv = pt[:, 512:1024]
                        nc.gpsimd.affine_select(
                            pv, pv, pattern=[[1, 512]],
                            compare_op=mybir.AluOpType.is_ge, fill=0.0,
                            base=-(128 * t), channel_multiplier=-1)
                    for half, op in ((0, op0), (1, op1)):
                        h = 2 * hp + half
                        nc.tensor.matmul(
                            op[0:65, :], vt[jt][:, h * VW:h * VW + 65],
                            pt[:, half * 512:(half + 1) * 512],
                            start=(jt == 0), stop=(jt == njt - 1))

                # stage unnormalized O + rowsums out of PSUM (fast release)
                cs = slice(c * 512, (c + 1) * 512)
                rs0 = rspool.tile([128, 512], F32, tag="rs", name="rs")
                rs1 = rspool.tile([128, 512], F32, tag="rs", name="rs")
                av0 = avpool.tile([128, 512], BF16, tag="av", name="av")
                av1 = avpool.tile([128, 512], BF16, tag="av", name="av")
                nc.vector.tensor_copy(av0[0:64, :], op0[0:64, :])
                nc.vector.tensor_copy(av0[64:65, :], op0[64:65, :])
                nc.vector.tensor_copy(rs0[0:1, :], op0[0:1, :])
                nc.vector.tensor_copy(av1[0:64, :], op1[0:64, :])
                nc.vector.tensor_copy(av1[64:65, :], op1[64:65, :])
                nc.vector.tensor_copy(rs1[0:1, :], op1[0:1, :])
                nc.sync.dma_start(oT[hp][0:64, cs], av0[1:65, :])
                nc.sync.dma_start(oT[hp][64:128, cs], av1[1:65, :])
                # rowsum broadcast via gpsimd (fast, no DMA round-trip)
                rv0 = rvpool.tile([128, 512], F32, tag="rv", name="rv")
                rv1 = rvpool.tile([128, 512], F32, tag="rv", name="rv")
                nc.gpsimd.partition_broadcast(rv0[:, :], rs0[0:1, :])
                nc.gpsimd.partition_broadcast(rv1[:, :], rs1[0:1, :])

                # run an EARLIER chunk's reciprocal+scale (deps long done)
                pending.append((hp, c, rv0, rv1))
                flush_pending(keep=2)
        flush_pending()

    nc.compile()
    return nc


def _shard_inputs(x, Wk, bk, Wq, bq, Wv, bv, Wp, bp):
    bf = ml_dtypes.bfloat16
    in_maps = []
    for core in range(NCORES):
        b, hg = core // 2, core % 2
        sl = slice(hg * D, (hg + 1) * D)
        in_maps.append({
            "xT": np.ascontiguousarray(x[b].T).astype(bf),
            "wq": np.ascontiguousarray(Wq[:, sl]).astype(bf),
            "wk": np.ascontiguousarray(Wk[:, sl]).astype(bf),
            "wv": np.ascontiguousarray(Wv[:, sl]).astype(bf),
            "wp": np.ascontiguousarray(Wp[sl, :]).astype(bf),
            "bq": np.ascontiguousarray(
                bq[sl].reshape(DT, 128).T).astype(np.float32),
            "bk": np.ascontiguousarray(
                bk[sl].reshape(DT, 128).T).astype(np.float32),
            "bvbc": np.ascontiguousarray(
                np.broadcast_to(bv[sl], (128, D))).astype(np.float32),
        })
    return in_maps


def kernel(x, Wk, bk, Wq, bq, Wv, bv, Wp, bp, _trace=False, _trace_kwargs=None):
    x, Wk, bk, Wq, bq, Wv, bv, Wp, bp = [
        np.asarray(a) for a in (x, Wk, bk, Wq, bq, Wv, bv, Wp, bp)]
    if "nc" not in _cache:
        _cache["nc"] = _build()
    nc = _cache["nc"]
    in_maps = _shard_inputs(x, Wk, bk, Wq, bq, Wv, bv, Wp, bp)
    kw = dict(_trace_kwargs or {})
    res = run_bass_kernel_spmd(nc, in_maps, core_ids=list(range(NCORES)),
                               trace=_trace, **kw)
    out = np.empty((B, T, C), np.float32)
    for b in range(B):
        yp = res.results[2 * b]["ypT"] + res.results[2 * b + 1]["ypT"]
        out[b] = yp.T + bp[None, :]
    if _trace:
        _cache["last_results"] = res
    return out
